# revision 1
# baseline (speedup 1.0000x reference)
"""BandSplit kernel for Trainium2, 8 NeuronCores, batch-parallel.

Math (per band k with nb bins, f = 2*nb features):
  x_k (B, T, f) = interleaved re/im of spec_ri band slice
  xn = (x - mean_f(x)) / sqrt(var_f(x) + eps)
  y[b,t,e] = sum_f (W[k,e,f]*gamma[k,f]) * xn[b,t,f]  + (W[k]@beta[k] + b[k])[e]
  out[b, e, k, t] = y[b, t, e]

Device layout (per core, one batch element):
  - bins live on SBUF partitions in natural order (bands 0..34 rows 0..960,
    band 35 moved to rows 1024..1089 so its 65-row matmul window starts at a
    group boundary); NP = 1152 rows = 9 groups of 128
  - per-band matmuls use 32-aligned windows; 16-bin bands share a 32-row
    window with their neighbor, whose rows carry zero weights
  - per-band mean/meansq via ones-mask matmuls accumulated over groups
    (PSUM accumulation also folds the re/im pair sum)
  - -mu and 1/std expanded back to bin rows via transposed-mask matmuls
  - normalize on vector engine, per-band matmul (E=128 partitions out),
    bias add on scalar engine, DMA out
"""

import numpy as np

BINS = [16] * 20 + [32] * 10 + [64] * 5 + [65]
K = 36
E = 128
T = 2048
B = 8
EPS = 1e-5
TC = 256          # time-chunk (columns of the output per step)
NCH = T // TC     # 8 chunks
N_CORES = 8


def _band_layout():
    """Per band: (row, wrow, kp) — data row, matmul-window row, window K."""
    out = []
    for k, nb in enumerate(BINS):
        if nb == 16:
            row = 16 * k
            wrow, kp = 32 * (k // 2), 32
        elif nb == 32:
            row = 320 + 32 * (k - 20)
            wrow, kp = row, 32
        elif nb == 64:
            row = 640 + 64 * (k - 30)
            wrow, kp = row, 64
        else:  # nb == 65
            row = 1024
            wrow, kp = 1024, 65
        out.append((row, wrow, kp))
    return out, 1152


_LAYOUT, NP = _band_layout()
NG = NP // 128  # 9 groups

_CACHE = {}
TRACE = False
LAST_RESULT = None


def _build_program(repeat=1, probe_dma=False, probe_empty=False):
    from contextlib import ExitStack

    import concourse.tile as tile
    import concourse.mybir as mybir
    from concourse import bacc

    dt = mybir.dt.float32
    dtr = mybir.dt.float32r  # full-rate PE mode (fp32 is 1/4 rate)
    Alu = mybir.AluOpType
    Act = mybir.ActivationFunctionType

    nc = bacc.Bacc("TRN2", target_bir_lowering=False, debug=False,
                   num_devices=N_CORES)

    x = nc.dram_tensor("x", [NP, 2 * T], dt, kind="ExternalInput").ap()
    w_re = nc.dram_tensor("w_re", [128, K * 128], dt, kind="ExternalInput").ap()
    w_im = nc.dram_tensor("w_im", [128, K * 128], dt, kind="ExternalInput").ap()
    mask = nc.dram_tensor("mask", [128, NG * K], dt, kind="ExternalInput").ap()
    maskT = nc.dram_tensor("maskT", [K, NP], dt, kind="ExternalInput").ap()
    cst = nc.dram_tensor("cst", [128, K], dt, kind="ExternalInput").ap()
    ninvf = nc.dram_tensor("ninvf", [K, 1], dt, kind="ExternalInput").ap()
    invf = nc.dram_tensor("invf", [K, 1], dt, kind="ExternalInput").ap()
    epsc = nc.dram_tensor("epsc", [K, 1], dt, kind="ExternalInput").ap()
    out = nc.dram_tensor("out", [128, K * T], dt, kind="ExternalOutput").ap()

    with tile.TileContext(nc) as tc, ExitStack() as ctx:
        cw = ctx.enter_context(tc.tile_pool(name="cw", bufs=1))
        xp = ctx.enter_context(tc.tile_pool(name="xp", bufs=22))
        x2p = ctx.enter_context(tc.tile_pool(name="x2p", bufs=4))
        xnp = ctx.enter_context(tc.tile_pool(name="xnp", bufs=22))
        sb = ctx.enter_context(tc.tile_pool(name="sb", bufs=2))
        op = ctx.enter_context(tc.tile_pool(name="op", bufs=6))
        statps = ctx.enter_context(tc.tile_pool(name="statps", bufs=1, space="PSUM"))
        expps = ctx.enter_context(tc.tile_pool(name="expps", bufs=2, space="PSUM"))
        pps = ctx.enter_context(tc.tile_pool(name="pps", bufs=4, space="PSUM"))

        wre_sb = cw.tile([128, K * 128], dtr)
        nc.sync.dma_start(out=wre_sb[:], in_=w_re[:].bitcast(dtr))
        wim_sb = cw.tile([128, K * 128], dtr)
        nc.sync.dma_start(out=wim_sb[:], in_=w_im[:].bitcast(dtr))
        mask_sb = cw.tile([128, NG * K], dtr)
        nc.sync.dma_start(out=mask_sb[:], in_=mask[:].bitcast(dtr))
        maskT_sb = cw.tile([K, NP], dtr)
        nc.sync.dma_start(out=maskT_sb[:], in_=maskT[:].bitcast(dtr))
        cst_sb = cw.tile([128, K], dt)
        nc.sync.dma_start(out=cst_sb[:], in_=cst[:])
        ninvf_sb = cw.tile([K, 1], dt)
        nc.sync.dma_start(out=ninvf_sb[:], in_=ninvf[:])
        invf_sb = cw.tile([K, 1], dt)
        nc.sync.dma_start(out=invf_sb[:], in_=invf[:])
        epsc_sb = cw.tile([K, 1], dt)
        nc.sync.dma_start(out=epsc_sb[:], in_=epsc[:])

        rep_cm = tc.For_i(0, repeat) if repeat > 1 else None
        if rep_cm is not None:
            rep_cm.__enter__()

        if probe_empty:
            o = op.tile([128, TC], dt)
            nc.scalar.copy(o[:], wre_sb[:, 0:TC])
            nc.sync.dma_start(out=out[:, 0:TC], in_=o[:])
            if rep_cm is not None:
                rep_cm.__exit__(None, None, None)
            nc.compile()
            return nc

        for c in range(NCH):
            col0 = c * 2 * TC
            # ---- load x group tiles ----
            xg = []
            for g in range(NG):
                t = xp.tile([128, 2 * TC], dtr)
                nc.sync.dma_start(
                    out=t[:],
                    in_=x[g * 128:(g + 1) * 128, col0:col0 + 2 * TC].bitcast(dtr))
                xg.append(t)

            if probe_dma:
                # DMA floor probe: skip compute, store copied tiles
                for k in range(K):
                    o = op.tile([128, TC], dt)
                    nc.scalar.copy(o[:], xg[k % NG][:, 0:TC])
                    nc.sync.dma_start(
                        out=out[:, k * T + c * TC: k * T + (c + 1) * TC],
                        in_=o[:])
                continue

            # ---- stats matmuls: S1 = sum x, S2 = sum x^2 per band ----
            s1p = statps.tile([K, 2 * TC], dt, tag="s1p")
            s2p = statps.tile([K, 2 * TC], dt, tag="s2p")
            for g in range(NG):
                x2 = x2p.tile([128, 2 * TC], dtr)
                nc.scalar.square(x2[:], xg[g][:])
                mg = mask_sb[:, g * K:(g + 1) * K]
                st, sp = (g == 0), (g == NG - 1)
                nc.tensor.matmul(s1p[:], mg, xg[g][:], start=st, stop=sp)
                nc.tensor.matmul(s2p[:], mg, x2[:], start=st, stop=sp)

            # ---- fold re/im pairs, compute -mu and 1/std (36, TC) ----
            # (DVE may read only one PSUM operand; bounce via SBUF on ACT)
            s1c = sb.tile([K, 2 * TC], dt, tag="s1c")
            s2c = sb.tile([K, 2 * TC], dt, tag="s2c")
            nc.scalar.copy(s1c[:], s1p[:])
            nc.scalar.copy(s2c[:], s2p[:])
            s1v = s1c[:].rearrange("p (t c) -> p t c", c=2)
            s2v = s2c[:].rearrange("p (t c) -> p t c", c=2)
            s1 = sb.tile([K, TC], dt, tag="s1")
            s2 = sb.tile([K, TC], dt, tag="s2")
            nc.vector.tensor_tensor(s1[:], s1v[:, :, 0], s1v[:, :, 1], op=Alu.add)
            nc.vector.tensor_tensor(s2[:], s2v[:, :, 0], s2v[:, :, 1], op=Alu.add)
            negmu = sb.tile([K, TC], dtr, tag="negmu")
            nc.vector.tensor_scalar(negmu[:], s1[:], ninvf_sb[:], None, Alu.mult)
            musq = sb.tile([K, TC], dt, tag="musq")
            nc.vector.tensor_tensor(musq[:], negmu[:], negmu[:], op=Alu.mult)
            var = sb.tile([K, TC], dt, tag="var")
            nc.vector.scalar_tensor_tensor(
                var[:], s2[:], invf_sb[:], musq[:],
                op0=Alu.mult, op1=Alu.subtract)
            std = sb.tile([K, TC], dt, tag="std")
            nc.scalar.activation(std[:], var[:], Act.Sqrt, bias=epsc_sb[:])
            istd = sb.tile([K, TC], dtr, tag="istd")
            with nc.allow_low_precision(reason="fp32r feed for PE full-rate"):
                nc.vector.reciprocal(istd[:], std[:])

            # ---- expand -mu, 1/std to bin rows; normalize ----
            xn = []
            for g in range(NG):
                mtg = maskT_sb[:, g * 128:(g + 1) * 128]
                ex = expps.tile([128, 2 * TC], dt, tag="ex")
                nc.tensor.matmul(ex[:, 0:TC], mtg, negmu[:], start=True, stop=True)
                nc.tensor.matmul(ex[:, TC:2 * TC], mtg, istd[:], start=True, stop=True)
                xv = xg[g][:].rearrange("p (t c) -> p t c", c=2)
                xnt = xnp.tile([128, 2 * TC], dtr)
                nc.vector.tensor_tensor(xnt[:, 0:TC], xv[:, :, 0], ex[:, 0:TC], op=Alu.add)
                nc.vector.tensor_tensor(xnt[:, 0:TC], xnt[:, 0:TC], ex[:, TC:2 * TC], op=Alu.mult)
                nc.vector.tensor_tensor(xnt[:, TC:2 * TC], xv[:, :, 1], ex[:, 0:TC], op=Alu.add)
                nc.vector.tensor_tensor(xnt[:, TC:2 * TC], xnt[:, TC:2 * TC], ex[:, TC:2 * TC], op=Alu.mult)
                xn.append(xnt)

            # ---- per-band matmul + bias; store band quads (4KB DMA rows) ----
            # out DRAM is chunk-major: col = c*K*TC + k*TC (host unscrambles)
            for p in range(K // 4):
                o2 = op.tile([128, 4 * TC], dt)
                for h in (0, 1, 2, 3):
                    k = 4 * p + h
                    row, wrow, kp = _LAYOUT[k]
                    g, off = wrow // 128, wrow % 128
                    tp = (off, 0)
                    P = pps.tile([128, TC], dt, tag="P")
                    nc.tensor.matmul(
                        P[:], wre_sb[off:off + kp, k * 128:(k + 1) * 128],
                        xn[g][off:off + kp, 0:TC],
                        start=True, stop=False, tile_position=tp)
                    nc.tensor.matmul(
                        P[:], wim_sb[off:off + kp, k * 128:(k + 1) * 128],
                        xn[g][off:off + kp, TC:2 * TC],
                        start=False, stop=True, tile_position=tp)
                    nc.scalar.activation(o2[:, h * TC:(h + 1) * TC], P[:],
                                         Act.Identity, bias=cst_sb[:, k:k + 1])
                nc.sync.dma_start(
                    out=out[:, c * K * TC + 4 * p * TC: c * K * TC + (4 * p + 4) * TC],
                    in_=o2[:])

        if rep_cm is not None:
            rep_cm.__exit__(None, None, None)

    nc.compile()
    return nc


def _host_tensors(gamma, beta, W, b):
    w_re = np.zeros((128, K * 128), np.float32)
    w_im = np.zeros((128, K * 128), np.float32)
    mask = np.zeros((128, NG * K), np.float32)
    maskT = np.zeros((K, NP), np.float32)
    cst = np.zeros((128, K), np.float32)
    invf = np.zeros((K, 1), np.float32)
    for k, nb in enumerate(BINS):
        f = 2 * nb
        row, wrow, kp = _LAYOUT[k]
        g, off = row // 128, row % 128
        loc = row - wrow          # offset of band rows inside matmul window
        woff = wrow % 128
        Wg = (W[k, :, :f] * gamma[k, :f][None, :]).astype(np.float32)  # (E, f)
        w_re[woff + loc:woff + loc + nb, k * 128:(k + 1) * 128] = Wg[:, 0::2].T
        w_im[woff + loc:woff + loc + nb, k * 128:(k + 1) * 128] = Wg[:, 1::2].T
        mask[off:off + nb, g * K + k] = 1.0
        maskT[k, row:row + nb] = 1.0
        cst[:, k] = W[k, :, :f] @ beta[k, :f] + b[k]
        invf[k, 0] = 1.0 / f
    return {
        "w_re": w_re, "w_im": w_im, "mask": mask, "maskT": maskT,
        "cst": cst, "invf": invf, "ninvf": -invf,
        "epsc": np.full((K, 1), EPS, np.float32),
    }


def _unscramble(flat):
    # flat (128, K*T) in chunk-major [c, k, TC] order -> (E, K, T)
    a = flat.reshape(E, NCH, K, TC)
    return np.ascontiguousarray(a.transpose(0, 2, 1, 3)).reshape(E, K, T)


def _pad_x(spec_ri_b):
    # spec_ri_b: (1025, 2048, 2) one batch element
    xpad = np.zeros((NP, 2 * T), np.float32)
    xpad[0:960] = spec_ri_b[0:960].reshape(960, 2 * T)
    xpad[1024:1089] = spec_ri_b[960:1025].reshape(65, 2 * T)
    return xpad


def kernel(spec_ri, gamma, beta, W, b):
    global LAST_RESULT
    from concourse.bass_utils import run_bass_kernel_spmd

    if "nc" not in _CACHE:
        _CACHE["nc"] = _build_program()
    nc = _CACHE["nc"]

    spec_ri = np.asarray(spec_ri, np.float32)
    consts = _host_tensors(np.asarray(gamma, np.float32),
                           np.asarray(beta, np.float32),
                           np.asarray(W, np.float32),
                           np.asarray(b, np.float32))
    in_maps = [{"x": _pad_x(spec_ri[i]), **consts} for i in range(N_CORES)]

    res = run_bass_kernel_spmd(nc, in_maps, core_ids=list(range(N_CORES)),
                               trace=TRACE)
    LAST_RESULT = res
    outs = [_unscramble(res.results[i]["out"]) for i in range(N_CORES)]
    return np.ascontiguousarray(np.stack(outs, axis=0))

